# revision 2
# baseline (speedup 1.0000x reference)
"""HINGCN edge-emb GNN on 8 trn2 cores — v4 (projected-table design).

Per metapath, precompute a node table tabP[m][node] =
[P1 = nemb@Wk1 (64) | P2 = nemb@Wk2 (64) | ku1 = nemb@Wk1@a1mid |
 ku2 = nemb@Wk2@a2mid | pad] in bf16 (132 payload cols, 136-col row
stride).  The main loop gathers projected 264-B rows instead of raw
512-B node features, which removes all per-tile projections/matmuls
and k-score dot products from the DVE, and halves gather bytes.

Gather strategy (V4_GATHER):
  "per_s": 32 single-offset indirect DMAs per (tile, metapath) —
           works everywhere (baseline semantics), Pool-bound.
  "multi": one indirect DMA with a [T, S] offset AP and a gapped
           source AP (payload 132 of 136-stride rows) — pending HW
           validation of per-run offset consumption.
"""

import math
import os
import sys

for _p in ("/opt/trn_rl_repo",):
    if _p not in sys.path:
        sys.path.insert(0, _p)

import numpy as np

import concourse.bacc as bacc
import concourse.mybir as mybir
from concourse.bass import IndirectOffsetOnAxis
from concourse.masks import make_identity
from concourse.tile import TileContext

F32 = mybir.dt.float32
BF16 = mybir.dt.bfloat16
I32 = mybir.dt.int32
AX = mybir.AxisListType
OP = mybir.AluOpType
ACT = mybir.ActivationFunctionType

NCORES = 8
T = 128
NB = 32
NFEAT = 128
NHID = 64
DIM_MP = 64
EDIM = 32
NMETA = 3
NCLASS = 8
ALPHA = 0.2
S = 32

N_NODES = 50000
NPAD = 50048           # 391 chunks of 128
NCHUNK = NPAD // 128   # 391
GRP = 8                # chunks per PSUM group
NGRP = (NCHUNK + GRP - 1) // GRP

CROW = 130             # table row stride == payload (dense rows)
CPAY = 130             # payload cols [P1|P2|ku1|ku2]
# table row order: node n -> row (n & 127) * NCHUNK + (n >> 7), so that
# build-group stores are per-partition contiguous runs

GATHER = "per_s"
CHAIN = "f32"
ERCAST = True   # cast-gather edge rows (f32 -> bf16 during SWDGE)


def build_nc(nt: int):
    nc = bacc.Bacc("TRN2", target_bir_lowering=False, debug=False)
    b_core = nt * T

    inp = nc.dram_tensor("inp", [b_core, NFEAT], F32, kind="ExternalInput").ap()
    idxd = nc.dram_tensor("idxd", [T, nt], I32, kind="ExternalInput").ap()
    nembT = nc.dram_tensor("nembT", [NFEAT, NPAD], BF16, kind="ExternalInput").ap()
    eid = [
        nc.dram_tensor(f"ei{m}", [N_NODES, NB], I32, kind="ExternalInput").ap()
        for m in range(NMETA)
    ]
    eed = [
        nc.dram_tensor(f"ee{m}", [N_NODES, NB * EDIM], F32, kind="ExternalInput").ap()
        for m in range(NMETA)
    ]
    wq1d = nc.dram_tensor("wq1", [NMETA, NFEAT, NHID], F32, kind="ExternalInput").ap()
    wk1d = nc.dram_tensor("wk1", [NMETA, NFEAT, NHID], F32, kind="ExternalInput").ap()
    a1d = nc.dram_tensor("a1", [NMETA, 2 * NHID + EDIM], F32, kind="ExternalInput").ap()
    wq2d = nc.dram_tensor("wq2", [NMETA, NHID, DIM_MP], F32, kind="ExternalInput").ap()
    wk2d = nc.dram_tensor("wk2", [NMETA, NFEAT, DIM_MP], F32, kind="ExternalInput").ap()
    a2d = nc.dram_tensor("a2", [NMETA, 2 * DIM_MP + EDIM], F32, kind="ExternalInput").ap()
    ampd = nc.dram_tensor("amp", [DIM_MP], F32, kind="ExternalInput").ap()
    wcd = nc.dram_tensor("wc", [DIM_MP, NCLASS], F32, kind="ExternalInput").ap()
    bcd = nc.dram_tensor("bc", [NCLASS], F32, kind="ExternalInput").ap()
    outd = nc.dram_tensor("outp", [b_core, NCLASS], F32, kind="ExternalOutput").ap()

    tabP = [
        nc.dram_tensor(f"tabP{m}", [NPAD, CPAY], BF16, kind="Internal").ap()
        for m in range(NMETA)
    ]

    with TileContext(nc) as tc:
        with (
            tc.tile_pool(name="persist", bufs=1) as pp,
            tc.tile_pool(name="prep", bufs=2) as prep,
            tc.tile_pool(name="slab", bufs=2) as slab_pool,
            tc.tile_pool(name="gpool", bufs=4) as gpool,
            tc.tile_pool(name="spool", bufs=2) as spool,
            tc.tile_pool(name="small", bufs=3) as sm,
            tc.tile_pool(name="psA", bufs=2, space="PSUM") as psA,
            tc.tile_pool(name="psB", bufs=2, space="PSUM") as psB,
        ):
            # ======== preamble ========
            ident = pp.tile([128, 128], F32, name="ident")
            make_identity(nc, ident[:])
            ones1 = pp.tile([1, 128], F32, name="ones1")
            nc.vector.memset(ones1[:], 1.0)

            def brow(row, width, name, dtype=F32):
                p = psB.tile([128, width], F32, tag="prep_ps", name=f"{name}_bp")
                nc.tensor.matmul(out=p[:], lhsT=ones1[:], rhs=row[0:1, :])
                t = pp.tile([128, width], dtype, name=name)
                nc.vector.tensor_copy(out=t[:], in_=p[:])
                return t

            idxs = pp.tile([T, nt], I32, name="idxs")
            nc.sync.dma_start(out=idxs[:], in_=idxd[:, :])

            WPACK = []      # [128, 128] bf16 = [Wk1 | Wk2]
            WKU = []        # [128, 2] bf16 = [u1 | u2]
            V2 = []         # [128, 64] f32
            AE1 = []        # [128, 32] bf16
            AE2 = []
            V1cols = pp.tile([NFEAT, NMETA], F32, name="V1cols")

            for m in range(NMETA):
                wk1_m = prep.tile([NFEAT, NHID], F32, tag="wk_m")
                nc.sync.dma_start(out=wk1_m[:], in_=wk1d[m])
                wk2_m = prep.tile([NFEAT, DIM_MP], F32, tag="wk2_m")
                nc.sync.dma_start(out=wk2_m[:], in_=wk2d[m])
                wq1_m = prep.tile([NFEAT, NHID], F32, tag="wq_m")
                nc.sync.dma_start(out=wq1_m[:], in_=wq1d[m])
                wq2_m = prep.tile([NHID, DIM_MP], F32, tag="wq2_m")
                nc.sync.dma_start(out=wq2_m[:], in_=wq2d[m])

                wp = pp.tile([NFEAT, 128], BF16, name=f"wpack_{m}")
                nc.vector.tensor_copy(out=wp[:, 0:NHID], in_=wk1_m[:])
                nc.vector.tensor_copy(out=wp[:, NHID : 2 * NHID], in_=wk2_m[:])
                WPACK.append(wp)
                wu = pp.tile([NFEAT, 2], BF16, name=f"wku_{m}")
                WKU.append(wu)

                a1lo = prep.tile([NHID, 1], F32, tag="alo")
                nc.sync.dma_start(out=a1lo[:], in_=a1d[m, 0:NHID, None])
                a1mid = prep.tile([NHID, 1], F32, tag="amid")
                nc.sync.dma_start(out=a1mid[:], in_=a1d[m, NHID : 2 * NHID, None])
                a2lo = prep.tile([DIM_MP, 1], F32, tag="a2lo")
                nc.sync.dma_start(out=a2lo[:], in_=a2d[m, 0:DIM_MP, None])
                a2mid = prep.tile([DIM_MP, 1], F32, tag="a2mid")
                nc.sync.dma_start(out=a2mid[:], in_=a2d[m, DIM_MP : 2 * DIM_MP, None])

                ae1r = prep.tile([1, EDIM], F32, tag="ae1r")
                nc.sync.dma_start(out=ae1r[:], in_=a1d[m, None, 2 * NHID :])
                ae2r = prep.tile([1, EDIM], F32, tag="ae2r")
                nc.sync.dma_start(out=ae2r[:], in_=a2d[m, None, 2 * DIM_MP :])
                AE1.append(brow(ae1r, EDIM, f"ae1b_{m}", BF16))
                AE2.append(brow(ae2r, EDIM, f"ae2b_{m}", BF16))

                def _tp(dst_shape, src, tag):
                    kk = src.shape[0]
                    p = psB.tile(
                        [dst_shape[0], dst_shape[1]], F32, tag="prep_ps", name="tp_ps"
                    )
                    nc.tensor.transpose(out=p[:], in_=src[:], identity=ident[0:kk, 0:kk])
                    t = prep.tile(dst_shape, F32, tag=tag)
                    nc.vector.tensor_copy(out=t[:], in_=p[:])
                    return t

                wk1t = _tp([NHID, NFEAT], wk1_m, "wk1t")
                wk2t = _tp([DIM_MP, NFEAT], wk2_m, "wk2t")
                wq1t = _tp([NHID, NFEAT], wq1_m, "wq1t")
                wq2t = _tp([DIM_MP, NHID], wq2_m, "wq2t")

                # u columns: u1 = Wk1 @ a1mid -> [128, 1]
                u1p = psB.tile([NFEAT, 1], F32, tag="prep_ps", name="u1_ps")
                nc.tensor.matmul(out=u1p[:], lhsT=wk1t[:], rhs=a1mid[:])
                nc.vector.tensor_copy(out=wu[:, 0:1], in_=u1p[:])
                u2p = psB.tile([NFEAT, 1], F32, tag="prep_ps", name="u2_ps")
                nc.tensor.matmul(out=u2p[:], lhsT=wk2t[:], rhs=a2mid[:])
                nc.vector.tensor_copy(out=wu[:, 1:2], in_=u2p[:])

                # v1 column: Wq1 @ a1_lo
                v1p = psB.tile([NFEAT, 1], F32, tag="prep_ps", name="vcol_ps")
                nc.tensor.matmul(out=v1p[:], lhsT=wq1t[:], rhs=a1lo[:])
                nc.vector.tensor_copy(out=V1cols[:, m : m + 1], in_=v1p[:])

                # v2 row: a2_lo^T @ Wq2T -> broadcast
                v2p = psB.tile([1, NHID], F32, tag="prep_ps", name="v2_ps")
                nc.tensor.matmul(out=v2p[:], lhsT=a2lo[:], rhs=wq2t[:])
                v2 = prep.tile([1, NHID], F32, tag="v2row")
                nc.vector.tensor_copy(out=v2[:], in_=v2p[:])
                V2.append(brow(v2, NHID, f"v2b_{m}"))

            ampr = prep.tile([1, DIM_MP], F32, tag="ampr")
            nc.sync.dma_start(out=ampr[:], in_=ampd[None, :])
            amp = brow(ampr, DIM_MP, "ampb")
            wc = pp.tile([DIM_MP, NCLASS], F32, name="wc")
            nc.sync.dma_start(out=wc[:], in_=wcd[:, :])
            bcr0 = prep.tile([1, NCLASS], F32, tag="bcr0")
            nc.sync.dma_start(out=bcr0[:], in_=bcd[None, :])
            bcr = brow(bcr0, NCLASS, "bcb")

            inputT = pp.tile([NFEAT, b_core], F32, name="inputT")
            Q1 = pp.tile([T, nt * NMETA], F32, name="Q1")
            for t in range(nt):
                itile = prep.tile([T, NFEAT], F32, tag="itile")
                nc.sync.dma_start(out=itile[:], in_=inp[t * T : (t + 1) * T, :])
                itp = psB.tile([NFEAT, T], F32, tag="prep_ps", name="itp_ps")
                nc.tensor.transpose(out=itp[:], in_=itile[:], identity=ident[:])
                nc.vector.tensor_copy(out=inputT[:, t * T : (t + 1) * T], in_=itp[:])
                q1p = psB.tile([T, NMETA], F32, tag="prep_ps", name="q1_ps")
                nc.tensor.matmul(
                    out=q1p[:], lhsT=inputT[:, t * T : (t + 1) * T], rhs=V1cols[:]
                )
                nc.vector.tensor_copy(out=Q1[:, t * NMETA : (t + 1) * NMETA], in_=q1p[:])

            x2all = pp.tile([T, nt * NMETA * DIM_MP], F32, name="x2all")
            OUTS = pp.tile([T, nt * NCLASS], F32, name="OUTS")
            junk = pp.tile([T, 64], F32, name="junk")

            # ======== table build: single nembT sweep, all 3 metapaths ====
            def emit_build_group(m, g):
                k0 = g * GRP
                kn = min(GRP, NCHUNK - k0)
                ncols = kn * 128
                slab = slab_pool.tile([NFEAT, GRP * 128], BF16, tag="slab")
                nc.sync.dma_start(
                    out=slab[:, 0:ncols], in_=nembT[:, k0 * 128 : k0 * 128 + ncols]
                )
                gps = psA.tile([128, GRP * 128], F32, tag="gps", name="gps")
                kps = psA.tile([128, GRP * 2], F32, tag="kps", name="kps")
                for k in range(kn):
                    nc.tensor.matmul(
                        out=gps[:, k * 128 : (k + 1) * 128],
                        lhsT=slab[:, k * 128 : (k + 1) * 128],
                        rhs=WPACK[m][:],
                    )
                    nc.tensor.matmul(
                        out=kps[:, k * 2 : (k + 1) * 2],
                        lhsT=slab[:, k * 128 : (k + 1) * 128],
                        rhs=WKU[m][:],
                    )
                sbg = slab_pool.tile([128, GRP * CPAY], BF16, tag="sbg")
                if (g + m) % 2 == 0:
                    cp = nc.scalar.copy
                else:
                    cp = lambda out, in_: nc.vector.tensor_copy(out=out, in_=in_)
                cp(
                    out=sbg[:, 0 : kn * CPAY].rearrange(
                        "p (k c) -> p k c", c=CPAY
                    )[:, :, 0:128],
                    in_=gps[:, 0 : kn * 128].rearrange("p (k c) -> p k c", c=128),
                )
                cp(
                    out=sbg[:, 0 : kn * CPAY].rearrange(
                        "p (k c) -> p k c", c=CPAY
                    )[:, :, 128:130],
                    in_=kps[:, 0 : kn * 2].rearrange("p (k c) -> p k c", c=2),
                )
                nc.sync.dma_start(
                    out=tabP[m].rearrange("(p r) c -> p r c", r=NCHUNK)[
                        :, k0 : k0 + kn, :
                    ],
                    in_=sbg[:, 0 : kn * CPAY].rearrange("p (k c) -> p k c", c=CPAY),
                )

            # ======== helpers ========
            def softmax_att(st, qcol, tag):
                sq = sm.tile([T, S], F32, tag=f"sq{tag}")
                nc.vector.tensor_scalar_add(out=sq[:], in0=st[:], scalar1=qcol)
                sl = sm.tile([T, S], F32, tag=f"sl{tag}")
                nc.vector.scalar_tensor_tensor(
                    out=sl[:], in0=sq[:], scalar=ALPHA, in1=sq[:],
                    op0=OP.mult, op1=OP.max,
                )
                ex = sm.tile([T, S], F32, tag=f"ex{tag}")
                ssum = sm.tile([T, 1], F32, tag=f"ssum{tag}")
                nc.scalar.activation(
                    out=ex[:], in_=sl[:], func=ACT.Exp, accum_out=ssum[:]
                )
                rec = sm.tile([T, 1], F32, tag=f"rec{tag}")
                nc.vector.reciprocal(out=rec[:], in_=ssum[:])
                att = sm.tile([T, S], F32, tag=f"att{tag}")
                nc.vector.tensor_scalar_mul(out=att[:], in0=ex[:], scalar1=rec[:, 0:1])
                return att

            def escore(erb, aeb, tag):
                me = spool.tile([T, S * EDIM], BF16, tag=f"me{tag}")
                nc.vector.tensor_tensor(
                    out=me[:].rearrange("p (s e) -> p s e", e=EDIM),
                    in0=erb[:].rearrange("p (s e) -> p s e", e=EDIM),
                    in1=aeb[:, None, :].to_broadcast([T, S, EDIM]),
                    op=OP.mult,
                )
                r = sm.tile([T, S], F32, tag=f"es{tag}")
                nc.vector.reduce_sum(
                    out=r[:],
                    in_=me[:].rearrange("p (s e) -> p s e", e=EDIM),
                    axis=AX.X,
                )
                return r

            def weighted_sum(G2, att, lo, tag):
                """ws[p,f] = sum_s att[p,s]*G2[p, s*CROW+lo : +64], f32 acc."""
                acc = [
                    sm.tile([T, 64], F32, tag=f"acc0{tag}", name="acc0"),
                    sm.tile([T, 64], F32, tag=f"acc1{tag}", name="acc1"),
                ]
                nc.vector.tensor_scalar_mul(
                    out=acc[0][:], in0=G2[:, lo : lo + 64], scalar1=att[:, 0:1]
                )
                for s in range(1, S):
                    src = acc[(s + 1) % 2]
                    dst = acc[s % 2]
                    nc.vector.scalar_tensor_tensor(
                        out=dst[:],
                        in0=G2[:, s * CROW + lo : s * CROW + lo + 64],
                        scalar=att[:, s : s + 1],
                        in1=src[:],
                        op0=OP.mult,
                        op1=OP.add,
                    )
                return acc[(S - 1) % 2]

            def elu(x, tag, out=None):
                rl = sm.tile([T, 64], F32, tag=f"elu_rl{tag}")
                nc.vector.tensor_scalar_max(out=rl[:], in0=x[:], scalar1=0.0)
                mn = sm.tile([T, 64], F32, tag=f"elu_mn{tag}")
                nc.vector.tensor_scalar_min(out=mn[:], in0=x[:], scalar1=0.0)
                exm = sm.tile([T, 64], F32, tag=f"elu_ex{tag}")
                nc.scalar.activation(out=exm[:], in_=mn[:], func=ACT.Exp)
                if out is None:
                    out = sm.tile([T, 64], F32, tag=f"elu_x{tag}")
                nc.vector.scalar_tensor_tensor(
                    out=out[:], in0=exm[:], scalar=-1.0, in1=rl[:],
                    op0=OP.add, op1=OP.add,
                )
                return out

            # ======== main loop (build table m, then run metapath m) ====
            def gather_nbr_er(m, t):
                nbrs = gpool.tile([T, NB], I32, tag="nbrs")
                nc.gpsimd.indirect_dma_start(
                    out=nbrs[:],
                    out_offset=None,
                    in_=eid[m][:, :],
                    in_offset=IndirectOffsetOnAxis(ap=idxs[:, t : t + 1], axis=0),
                )
                erb = gpool.tile([T, NB * EDIM], BF16 if ERCAST else F32, tag="erb")
                nc.gpsimd.indirect_dma_start(
                    out=erb[:],
                    out_offset=None,
                    in_=eed[m][:, :],
                    in_offset=IndirectOffsetOnAxis(ap=idxs[:, t : t + 1], axis=0),
                )
                if not ERCAST:
                    erc = spool.tile([T, NB * EDIM], BF16, tag="erc")
                    nc.vector.tensor_copy(out=erc[:], in_=erb[:])
                    erb = erc
                return nbrs, erb

            pend = None
            for m in range(NMETA):
                for g in range(NGRP):
                    emit_build_group(m, g)
                if m == 0:
                    pend = gather_nbr_er(0, 0)
                for t in range(nt):
                    nbrs, erb = pend
                    nxt = (m, t + 1) if t + 1 < nt else (m + 1, 0)
                    if nxt[0] < NMETA:
                        pend = gather_nbr_er(*nxt)

                    nbr2 = nbrs
                    G2 = gpool.tile([T, S * CROW], BF16, tag="G2")
                    for s_ in range(S):
                        nc.gpsimd.indirect_dma_start(
                            out=G2[:, s_ * CROW : s_ * CROW + CPAY],
                            out_offset=None,
                            in_=tabP[m][:, :],
                            in_offset=IndirectOffsetOnAxis(
                                ap=nbr2[:, s_ : s_ + 1], axis=0
                            ),
                        )

                    # k-score extraction (cols 128, 129 of each row)
                    k1 = sm.tile([T, S], F32, tag="k1")
                    nc.vector.tensor_copy(
                        out=k1[:].rearrange("p (s o) -> p s o", o=1),
                        in_=G2[:].rearrange("p (s c) -> p s c", c=CROW)[:, :, 128:129],
                    )
                    k2 = sm.tile([T, S], F32, tag="k2")
                    nc.vector.tensor_copy(
                        out=k2[:].rearrange("p (s o) -> p s o", o=1),
                        in_=G2[:].rearrange("p (s c) -> p s c", c=CROW)[:, :, 129:130],
                    )

                    # layer 1
                    e1 = escore(erb, AE1[m], "1")
                    st1 = sm.tile([T, S], F32, tag="st1")
                    nc.vector.tensor_add(out=st1[:], in0=k1[:], in1=e1[:])
                    att1 = softmax_att(
                        st1, Q1[:, t * NMETA + m : t * NMETA + m + 1], "1"
                    )
                    ws1 = weighted_sum(G2, att1, 0, "1")
                    x1 = elu(ws1, "1")
                    q2 = sm.tile([T, 1], F32, tag="q2c")
                    nc.vector.scalar_tensor_tensor(
                        out=junk[:], in0=x1[:], scalar=1.0, in1=V2[m][:],
                        op0=OP.mult, op1=OP.mult, accum_out=q2[:],
                    )

                    # layer 2
                    e2 = escore(erb, AE2[m], "2")
                    st2 = sm.tile([T, S], F32, tag="st2")
                    nc.vector.tensor_add(out=st2[:], in0=k2[:], in1=e2[:])
                    att2 = softmax_att(st2, q2[:, 0:1], "2")
                    ws2 = weighted_sum(G2, att2, 64, "2")
                    elu(
                        ws2, "2",
                        out=x2all[
                            :, (t * NMETA + m) * DIM_MP : (t * NMETA + m + 1) * DIM_MP
                        ],
                    )

            # ======== fusion + classifier ========
            for t in range(nt):
                x2s = x2all[:, t * NMETA * DIM_MP : (t + 1) * NMETA * DIM_MP]
                fsc = sm.tile([T, NMETA], F32, tag="fsc")
                for m in range(NMETA):
                    fm = sm.tile([T, 1], F32, tag="fm")
                    nc.vector.scalar_tensor_tensor(
                        out=junk[:],
                        in0=x2s[:, m * DIM_MP : (m + 1) * DIM_MP],
                        scalar=1.0,
                        in1=amp[:],
                        op0=OP.mult,
                        op1=OP.mult,
                        accum_out=fm[:],
                    )
                    nc.vector.tensor_copy(out=fsc[:, m : m + 1], in_=fm[:])
                fl = sm.tile([T, NMETA], F32, tag="fl")
                nc.vector.scalar_tensor_tensor(
                    out=fl[:], in0=fsc[:], scalar=ALPHA, in1=fsc[:],
                    op0=OP.mult, op1=OP.max,
                )
                fex = sm.tile([T, NMETA], F32, tag="fex")
                fsum = sm.tile([T, 1], F32, tag="fsum")
                nc.scalar.activation(
                    out=fex[:], in_=fl[:], func=ACT.Exp, accum_out=fsum[:]
                )
                frec = sm.tile([T, 1], F32, tag="frec")
                nc.vector.reciprocal(out=frec[:], in_=fsum[:])
                attm = sm.tile([T, NMETA], F32, tag="attm")
                nc.vector.tensor_scalar_mul(
                    out=attm[:], in0=fex[:], scalar1=frec[:, 0:1]
                )

                fused = [
                    sm.tile([T, DIM_MP], F32, tag="fused0", name="fused0"),
                    sm.tile([T, DIM_MP], F32, tag="fused1", name="fused1"),
                ]
                nc.vector.tensor_scalar_mul(
                    out=fused[0][:], in0=x2s[:, 0:DIM_MP], scalar1=attm[:, 0:1]
                )
                for m in range(1, NMETA):
                    nc.vector.scalar_tensor_tensor(
                        out=fused[m % 2][:],
                        in0=x2s[:, m * DIM_MP : (m + 1) * DIM_MP],
                        scalar=attm[:, m : m + 1],
                        in1=fused[(m + 1) % 2][:],
                        op0=OP.mult,
                        op1=OP.add,
                    )
                fin = fused[(NMETA - 1) % 2]

                ftp = psB.tile([DIM_MP, T], F32, tag="prep_ps", name="ftp")
                nc.tensor.transpose(out=ftp[:], in_=fin[:], identity=ident[:])
                fts = sm.tile([DIM_MP, T], F32, tag="fts")
                nc.vector.tensor_copy(out=fts[:], in_=ftp[:])
                lg = psB.tile([T, NCLASS], F32, tag="prep_ps", name="lg")
                nc.tensor.matmul(out=lg[:], lhsT=fts[:], rhs=wc[:])
                lb = sm.tile([T, NCLASS], F32, tag="lb")
                nc.vector.tensor_tensor(out=lb[:], in0=lg[:], in1=bcr[:, :], op=OP.add)
                lr = sm.tile([T, NCLASS], F32, tag="lr")
                nc.vector.tensor_scalar_max(out=lr[:], in0=lb[:], scalar1=0.0)

                mx = sm.tile([T, 1], F32, tag="mx")
                nc.vector.reduce_max(out=mx[:], in_=lr[:], axis=AX.X)
                sh = sm.tile([T, NCLASS], F32, tag="sh")
                nc.vector.tensor_scalar_sub(out=sh[:], in0=lr[:], scalar1=mx[:, 0:1])
                shex = sm.tile([T, NCLASS], F32, tag="shex")
                sesum = sm.tile([T, 1], F32, tag="sesum")
                nc.scalar.activation(
                    out=shex[:], in_=sh[:], func=ACT.Exp, accum_out=sesum[:]
                )
                lse = sm.tile([T, 1], F32, tag="lse")
                nc.scalar.activation(out=lse[:], in_=sesum[:], func=ACT.Ln)
                nc.vector.tensor_scalar_sub(
                    out=OUTS[:, t * NCLASS : (t + 1) * NCLASS],
                    in0=sh[:],
                    scalar1=lse[:, 0:1],
                )

            nc.sync.dma_start(
                out=outd.rearrange("(t p) c -> p t c", p=T),
                in_=OUTS[:].rearrange("p (t c) -> p t c", c=NCLASS),
            )

    nc.compile()
    return nc


# ======== host side ========

_NC_CACHE: dict = {}
LAST_RESULTS = None


def _get_nc(nt):
    key = (nt, GATHER, CHAIN, ERCAST)
    if key not in _NC_CACHE:
        _NC_CACHE[key] = build_nc(nt)
    return _NC_CACHE[key]


def kernel(
    input,
    index,
    node_emb,
    edge_index,
    edge_emb,
    n_sample,
    Wq1,
    Wk1,
    a1,
    Wq2,
    Wk2,
    a2,
    a_mp,
    Wc,
    bc,
):
    import ml_dtypes
    from concourse.bass_utils import run_bass_kernel_spmd

    input = np.asarray(input, dtype=np.float32)
    index = np.asarray(index).astype(np.int64)
    node_emb = np.asarray(node_emb, dtype=np.float32)
    edge_index = np.asarray(edge_index, dtype=np.int32)
    edge_emb = np.asarray(edge_emb, dtype=np.float32)
    Sn = int(n_sample)
    assert Sn == S, f"kernel specialized to n_sample={S}, got {Sn}"

    B = input.shape[0]
    assert node_emb.shape[0] == N_NODES
    per = int(math.ceil(B / (NCORES * T))) * T
    nt = per // T
    b_pad = per * NCORES

    inp_p = np.zeros((b_pad, NFEAT), np.float32)
    inp_p[:B] = input
    idx_p = np.zeros((b_pad,), np.int64)
    idx_p[:B] = index

    nembT = np.zeros((NFEAT, NPAD), dtype=ml_dtypes.bfloat16)
    nembT[:, :N_NODES] = node_emb.T.astype(ml_dtypes.bfloat16)

    ee3 = edge_emb.reshape(NMETA, N_NODES, NB * EDIM)

    common = {
        "nembT": nembT,
        "wq1": np.asarray(Wq1, np.float32),
        "wk1": np.asarray(Wk1, np.float32),
        "a1": np.asarray(a1, np.float32),
        "wq2": np.asarray(Wq2, np.float32),
        "wk2": np.asarray(Wk2, np.float32),
        "a2": np.asarray(a2, np.float32),
        "amp": np.asarray(a_mp, np.float32),
        "wc": np.asarray(Wc, np.float32),
        "bc": np.asarray(bc, np.float32),
    }
    for m in range(NMETA):
        ei_rows = (edge_index[m] & 127) * NCHUNK + (edge_index[m] >> 7)
        common[f"ei{m}"] = np.ascontiguousarray(ei_rows.astype(np.int32))
        common[f"ee{m}"] = np.ascontiguousarray(ee3[m])

    in_maps = []
    for c in range(NCORES):
        sl = slice(c * per, (c + 1) * per)
        im = dict(common)
        im["inp"] = np.ascontiguousarray(inp_p[sl])
        im["idxd"] = np.ascontiguousarray(
            idx_p[sl].astype(np.int32).reshape(nt, T).T
        )
        in_maps.append(im)

    nc = _get_nc(nt)
    res = run_bass_kernel_spmd(nc, in_maps, core_ids=list(range(NCORES)))
    global LAST_RESULTS
    LAST_RESULTS = res
    out = np.concatenate([res.results[c]["outp"] for c in range(NCORES)], axis=0)
    return out[:B].astype(np.float32)
